# revision 105
# baseline (speedup 1.0000x reference)
"""Trainium2 Bass kernel for nn_Attention_41085657153633.

B=8, N=1024, C=384, H=6, D=64 attention with per-head q/k normalization
(mean/std over head_dim, ddof=1), softmax(QK^T/sqrt(D)) @ V, output proj.

Sharding: data-parallel over B — each of the 8 NeuronCores handles one
batch element end-to-end (no collectives).

Per-core dataflow (all matmul operands bf16; fp32 PSUM accumulation):
  - host supplies x[b]^T, qkv_w^T, proj_w^T pre-cast to bf16 (halves HBM
    traffic, enables fast weight load on the PE).
  - QKV^T computed head-major for Q,K ([d on partitions, tokens on free]
    — the layout QK^T wants) and token-major for V (the AV stationary).
    V is stored as [v_h | 64 ones cols] per head so AV rows 64-127
    accumulate the softmax denominator broadcast across 64 partitions
    for free in the matmul M dimension.
  - q/k normalization over the partition (d) axis via block-diagonal
    ones matmuls: mean_bcast = blockdiag(1/64) @ q and ssq_bcast =
    blockdiag(1/64) @ (q-mean)^2 come out broadcast across each head's
    64 partitions by construction; rstd = exp(-0.5*ln(ssq) + b) with the
    ddof=1 correction folded into the ACT bias (ln/exp share one ACT
    table set with the softmax exp — zero table switches after load).
  - scores S^T[s,t] per head via K^T x Q^T (contraction over d=64);
    head pairs occupy PE row groups T0/T8, alternating so each weight
    load overlaps the other group's matmul.
  - softmax denominators inverted as exp(-ln d) on ACT (ln/exp/softmax
    exp share one table set — no table switches in the whole kernel).
  - scheduling: the norm pipeline is software-pipelined into the QKV/V
    matmul stream (pair-0 QK^T already slots into the last chunk
    iterations), and one global QK/AV pipeline runs across all pairs
    with AV lagging 4 token-chunks behind QK and emitted BEFORE QK per
    iteration (keeps ready matmuls ahead of the s-psum WAR stall when
    the pair-end denominator ln/exp ops spike the ACT backlog). ACT is
    the phase-C metronome (~80us of exp/ln); per-head denominator
    chains are emitted right after that head's last AV matmul so the
    "av" psum bufs free one at a time. proj contractions k=0,1
    pre-accumulate after the AV stream, overlapping the tail exps +
    pair-2 denominators. PE idle gaps reset the p-state ramp (0.65/1.2
    GHz until 3us continuous busy, 2.4GHz after), so the stream order
    is tuned to keep the PE queue head unblocked: QKV/V before the
    chained mean/ssq per phase-B iteration, warmup matmuls covering
    the input-DMA window.
  - input DMA: qkv weights host-packed into consumption-ordered
    contiguous groups (one contiguous run per partition per DMA; the
    naive strided column slices produced 256B packets), x^T split
    per contraction chunk, outputs written back per 512-col half on
    alternating queues.
  - qk_ps -> SBUF copies run on ACT (Copy lives in every table set);
    ACT otherwise idles until the first norm ln at ~21us.
  - softmax max-subtraction skipped: |S/8| <~ 7, exp stays in range.
  - output proj from attn_out^T; bf16 result DMA'd out, host casts.

Measured on trn2 (ntff profile): 121.0us vs the 148.0/130.4us session
baseline. Engine budget: PE matmul ~87us union-busy (p-state sensitive),
ACT ~81us (48 softmax-exp tiles at 1.07ns/col + norm/denominator
ln/exp), DVE ~50us, Pool idle. Paths measured and rejected: DVE
InstReciprocal for denominators (6.4ns/col on HW), fp8e4 DoubleRow for
QK^T/AV (2x PE rate but 2.2-3.9e-2 rel err, over the 2e-2 gate),
token-major K + folded per-key exp scale (ACT -6.4us but +8us PE/DVE at
phase-B clocks), custom-DVE ops and gpsimd divide/PSUM access (rejected
by this walrus build).
"""

import sys

sys.path.insert(0, "/opt/trn_rl_repo")

import json

import numpy as np

B, N, C = 8, 1024, 384
H, D = 6, 64
NCORES = 8

_prog = None


def _install_multiwait_fixup():
    """This container's walrus build rejects >1 sync wait per instruction
    ("Too many sync wait commands"). Rewrite the BIR JSON before compile:
    hoist extra waits onto single-wait EventSemaphore instructions
    inserted just before the owner on the same engine (engines dispatch
    in program order, so the gating is preserved)."""
    from concourse import bass2jax, bass_utils

    if getattr(bass_utils, "_multiwait_fixup", False):
        return
    bass_utils._multiwait_fixup = True

    orig = bass_utils.compile_bir_kernel

    def _split(bir_json: bytes) -> bytes:
        j = json.loads(bir_json)
        for fn in j.get("functions", []):
            for bb in fn.get("blocks", []):
                out = []
                for inst in bb.get("instructions", []):
                    si = inst.get("sync_info")
                    waits = si.get("on_wait", []) if si else []
                    if len(waits) > 1:
                        for k, w in enumerate(waits[:-1]):
                            out.append({
                                "debug": inst.get("debug", 0),
                                "engine": inst["engine"],
                                "ins": [],
                                "outs": [],
                                "name": f"{inst['name']}-sw{k}",
                                "opcode": "EventSemaphore",
                                "sync_info": {"on_update": [], "on_wait": [w]},
                            })
                        si["on_wait"] = [waits[-1]]
                    out.append(inst)
                bb["instructions"] = out
        return json.dumps(j).encode()

    def patched(bir_json, tmpdir, neff_name="file.neff"):
        return orig(_split(bir_json), tmpdir, neff_name)

    bass_utils.compile_bir_kernel = patched
    bass2jax.compile_bir_kernel = patched
    # (note: --enable-ldw-opt=true was tried and is rejected by this
    # walrus build on our BIR — keep the default false)


def _build():
    import concourse.bass as bass
    import concourse.tile as tile
    from concourse import mybir

    _install_multiwait_fixup()

    F32 = mybir.dt.float32
    BF16 = mybir.dt.bfloat16
    EXP = mybir.ActivationFunctionType.Exp
    LN = mybir.ActivationFunctionType.Ln
    COPY = mybir.ActivationFunctionType.Copy
    SQUARE = mybir.ActivationFunctionType.Square

    nc = bass.Bass("TRN2")
    xT = nc.dram_tensor("xT", [C, N], BF16, kind="ExternalInput")
    # qkv weights host-packed into consumption-ordered contiguous groups
    # [c0|c3|V|c1|c4|c2|c5] so each input DMA moves one contiguous run per
    # partition (the old strided column slices produced 256B DMA packets)
    wqp = nc.dram_tensor("wqp", [128, 3456], BF16, kind="ExternalInput")
    proj_wT = nc.dram_tensor("proj_wT", [C, C], BF16, kind="ExternalInput")
    pb = nc.dram_tensor("pb", [128, 3], F32, kind="ExternalInput")
    bd_in = nc.dram_tensor("bd_in", [128, 128], BF16, kind="ExternalInput")
    outT = nc.dram_tensor("outT", [C, N], BF16, kind="ExternalOutput")

    KC = C // 128   # 3 contraction chunks of the model dim
    TC = N // 128   # 8 token chunks
    scale = float(D) ** -0.5
    order = [0, 3, 1, 4, 2, 5]   # q0,k0,q1,k1,q2,k2 chunk emission order

    with tile.TileContext(nc) as tc:
      with nc.allow_low_precision(reason="bf16 matmul intermediates"):
        with tc.tile_pool(name="consts", bufs=1) as consts, \
             tc.tile_pool(name="ins", bufs=1) as ins, \
             tc.tile_pool(name="persist", bufs=1) as persist, \
             tc.tile_pool(name="work", bufs=2) as work, \
             tc.tile_pool(name="es", bufs=16) as esp, \
             tc.tile_pool(name="avn", bufs=2) as avn, \
             tc.tile_pool(name="po", bufs=3) as pop, \
             tc.tile_pool(name="ps", bufs=2, space="PSUM") as ps:

            # ---- constants + input DMA ----
            bd = consts.tile([128, 128], BF16)
            pbt = consts.tile([128, 3], F32)
            warm_w = consts.tile([128, 64], BF16)
            ddof_b = consts.tile([128, 1], F32)
            warm_act = consts.tile([128, 1], F32)
            nc.vector.memset(warm_w[:], 0.5)
            nc.vector.memset(ddof_b[:], -0.5 * float(np.log(64.0 / 63.0)))

            xt = ins.tile([128, KC, N], BF16)
            wq = ins.tile([128, 3456], BF16)
            wp = ins.tile([128, KC, C], BF16)
            # group offsets in the packed wq layout
            OFFC = {0: 0, 3: 384, 1: 1920, 4: 2304, 2: 2688, 5: 3072}
            OFFV = 768

            # HAM warmup gated only on the tiny ddof memset — no DMA
            # dependency, so the PE clock gate starts opening immediately.
            warm_ps = ps.tile([64, 512], F32, tag="s")
            warm_rhs = bass.AP(
                tensor=warm_w.tensor, offset=warm_w.offset,
                ap=[list(warm_w.ap[0]), [0, 8], [1, 64]])  # [128,8,64] step-0
            for _ in range(12):
                nc.tensor.matmul(warm_ps[:], warm_w[:], warm_rhs,
                                 start=True, stop=True)

            xr = xT.rearrange("(k p) n -> p k n", p=128)
            # the first QKV matmul needs xt k0 AND the c0 weight group:
            # put each at the head of a different DMA queue so they land
            # in parallel as early as possible. Remaining chunks follow in
            # consumption order, all host-packed contiguous.
            nc.sync.dma_start(out=xt[:, 0, :], in_=xr[:, 0, :])
            nc.scalar.dma_start(out=wq[:, 0:384], in_=wqp[:, 0:384])    # c0
            nc.scalar.dma_start(out=xt[:, 1, :], in_=xr[:, 1, :])
            nc.scalar.dma_start(out=xt[:, 2, :], in_=xr[:, 2, :])
            # dummy exp AFTER the DMA triggers (same Scalar queue): pulls
            # the single ln/exp ACT table load into the input-DMA wait
            # without delaying the DMA kickoff
            nc.scalar.activation(warm_act[:], ddof_b[:], EXP)
            nc.sync.dma_start(out=wq[:, 384:768], in_=wqp[:, 384:768])  # c3
            nc.sync.dma_start(out=bd[:], in_=bd_in[:, :])
            nc.sync.dma_start(out=wq[:, 768:1920], in_=wqp[:, 768:1920])  # V
            nc.sync.dma_start(out=pbt[:], in_=pb[:, :])
            nc.sync.dma_start(out=wq[:, 1920:3456], in_=wqp[:, 1920:3456])
            nc.sync.dma_start(
                out=wp[:], in_=proj_wT.rearrange("(k p) m -> p k m", p=128))

            vo = persist.tile([128, TC, H, 128], BF16)
            nc.vector.memset(vo[:, :, :, D:128], 1.0)

            qn = persist.tile([128, 2 * H, N], BF16)   # normalized q | raw k
            aoT = persist.tile([128, KC, N], BF16)     # attn out (proj moving)

            # ---- phase B emitters: QKV chunks + pipelined normalization ----
            qk_ps = {}      # chunk j -> psum tile with raw q|k rows
            qkr = {}        # chunk j -> bf16 copy in SBUF
            mean_ps = {}
            qc = {}
            qc2 = {}
            ssq_ps = {}

            def emit_qkv_mm(j):
                p = ps.tile([128, N], F32, tag="s", name=f"qk_ps{j}")
                qk_ps[j] = p
                for k in range(KC):
                    for h5 in range(2):
                        nc.tensor.matmul(
                            p[:, h5 * 512:(h5 + 1) * 512],
                            wq[:, OFFC[j] + k * 128:OFFC[j] + (k + 1) * 128],
                            xt[:, k, h5 * 512:(h5 + 1) * 512],
                            start=(k == 0), stop=(k == KC - 1))

            def emit_qkv_copy(j):
                # PSUM->SBUF bf16 copy. First three chunks (c0,c3,c1) on
                # ACT: it idles ~10us at the start of phase B while DVE is
                # loaded there (Copy is in every ACT table set). Last three
                # on DVE: by then ACT is dense and those copies would queue
                # ahead of the pair-0-critical lnv/rstd ops that gate the
                # softmax exp stream start, while DVE has regained slack.
                q = work.tile([128, N], BF16, tag="qkr", name=f"qkr{j}")
                if j in (0, 3, 1):
                    nc.scalar.activation(q[:], qk_ps[j][:], COPY)
                else:
                    nc.vector.tensor_copy(q[:], qk_ps[j][:])
                qkr[j] = q

            def emit_v(t):
                v_ps = ps.tile([128, C], F32, tag="s", name=f"v_ps{t}")
                for k in range(KC):
                    nc.tensor.matmul(
                        v_ps[:],
                        xt[:, k, t * 128:(t + 1) * 128],
                        wq[:, OFFV + k * 384:OFFV + (k + 1) * 384],
                        start=(k == 0), stop=(k == KC - 1))
                nc.vector.tensor_copy(
                    vo[:, t, :, 0:D],
                    v_ps[:].rearrange("p (h d) -> p h d", h=H))

            def emit_mean(j):
                """mean broadcast + centered q, squared (DVE chain)."""
                m = ps.tile([128, N], F32, tag="av", name=f"mean_ps{j}")
                mean_ps[j] = m
                for h5 in range(2):
                    nc.tensor.matmul(m[:, h5 * 512:(h5 + 1) * 512],
                                     bd[:], qkr[j][:, h5 * 512:(h5 + 1) * 512],
                                     start=True, stop=True)
                c = work.tile([128, N], BF16, tag="qc", name=f"qc{j}")
                nc.vector.tensor_sub(c[:], qkr[j][:], m[:])
                qc[j] = c
                c2 = work.tile([128, N], BF16, tag="qc2", name=f"qc2{j}")
                nc.vector.tensor_mul(c2[:], c[:], c[:])
                qc2[j] = c2

            def emit_ssq(j):
                """ssq broadcast -> rstd (ACT ln/exp; Rsqrt is blocked in
                this bass build) -> qn (DVE mul)."""
                s = ps.tile([128, N], F32, tag="av", name=f"ssq_ps{j}")
                ssq_ps[j] = s
                for h5 in range(2):
                    nc.tensor.matmul(s[:, h5 * 512:(h5 + 1) * 512],
                                     bd[:], qc2[j][:, h5 * 512:(h5 + 1) * 512],
                                     start=True, stop=True)
                lnv = work.tile([128, N], F32, tag="lnv", name=f"lnv{j}")
                nc.scalar.activation(lnv[:], s[:], LN)
                rstd = work.tile([128, N], BF16, tag="rstd", name=f"rstd{j}")
                nc.scalar.activation(rstd[:], lnv[:], EXP, scale=-0.5,
                                     bias=ddof_b[:])
                nc.vector.tensor_mul(qn[:, j, :], qc[j][:], rstd[:])

            es_tiles = {}   # (pair, t, p) -> bf16 exp(scores) tile

            def emit_qk_t(j, t):
                """Scores S^T for heads 2j,2j+1, token chunk t (64-row PE
                mode, groups T0/T8 alternating) + the exp on ACT."""
                s_tiles = {}
                for p in range(2):
                    s_tiles[p] = ps.tile([128, N], F32, tag="s",
                                         name=f"s{j}_{t}_{p}")
                for h5 in range(2):
                    for p in range(2):
                        lo = p * 64
                        nc.tensor.matmul(
                            s_tiles[p][:, h5 * 512:(h5 + 1) * 512],
                            qn[lo:lo + 64, 3 + j, t * 128:(t + 1) * 128],
                            qn[lo:lo + 64, j, h5 * 512:(h5 + 1) * 512],
                            start=True, stop=True)
                for p in range(2):
                    es = esp.tile([128, N], BF16, tag="es",
                                  name=f"es{j}_{t}_{p}")
                    nc.scalar.activation(es[:], s_tiles[p][:], EXP, scale=scale)
                    es_tiles[(j, t, p)] = es

            av_ps = {}
            av_src = {}

            def emit_av_t(j, t):
                """AV accumulation step t for pair j (128-row mode). On the
                final step, each head's denominator/scale chain is emitted
                right after that head's last matmul so av_ps bufs free one
                at a time (halves the pair-boundary PE stall on the "av"
                tag WAR)."""
                for p in range(2):
                    if t == 0:
                        av_ps[(j, p)] = ps.tile([128, N], F32, tag="av",
                                                name=f"av{j}_{p}")
                    for h5 in range(2):
                        nc.tensor.matmul(
                            av_ps[(j, p)][:, h5 * 512:(h5 + 1) * 512],
                            vo[:, t, 2 * j + p, :],
                            es_tiles[(j, t, p)][:, h5 * 512:(h5 + 1) * 512],
                            start=(t == 0), stop=(t == TC - 1))
                    if t == TC - 1 and j < 2:
                        emit_norm_out_p(j, p)
                if t == TC - 1 and j == 2:
                    emit_norm_out_tail()

            def emit_norm_out_p(j, p):
                """Softmax denominator reciprocal. Rows 64-127 hold the
                denominator already broadcast across 64 partitions (ones
                columns of vo).

                Pairs 0,1: DVE InstReciprocal (6.4ns/col — slow, but fully
                off the critical path: the avc copy already freed the psum,
                and aoT is only needed by the proj tail). Putting these on
                ACT instead injects 4.2us into the ACT exp backlog, which
                starves the PE's QK stream mid-flight (measured in sim).
                Pair 2: ACT ln/exp — at the kernel tail ACT is idle and its
                2.2us latency beats DVE's 6.5us."""
                a = av_src.get((j, p), av_ps[(j, p)])
                rec = avn.tile([64, N], F32, tag="rec", name=f"rec{j}{p}")
                lnd = avn.tile([64, N], F32, tag="lnd", name=f"lnd{j}{p}")
                lo = p * 64
                nc.scalar.activation(lnd[:], a[D:128, :], LN)
                nc.scalar.activation(rec[:], lnd[:], EXP, scale=-1.0)
                nc.vector.tensor_mul(aoT[lo:lo + 64, j, :],
                                     a[0:D, :], rec[:])

            def emit_norm_out_tail():
                """Pair 2 closes the kernel: both heads' denominator chains
                per 512-col half, half-major order, so the k=2 proj matmuls
                and po2 bias (already h5/lo-split) start on the first half
                while ACT still processes the second."""
                recs = {}
                lnds = {}
                for p in range(2):
                    recs[p] = avn.tile([64, N], F32, tag="rec",
                                       name=f"rec2{p}")
                    lnds[p] = avn.tile([64, N], F32, tag="lnd",
                                       name=f"lnd2{p}")
                for h5 in range(2):
                    sl = slice(h5 * 512, (h5 + 1) * 512)
                    for p in range(2):
                        a = av_ps[(2, p)]
                        nc.scalar.activation(lnds[p][:, sl], a[D:128, sl], LN)
                        nc.scalar.activation(recs[p][:, sl], lnds[p][:, sl],
                                             EXP, scale=-1.0)
                        nc.vector.tensor_mul(aoT[p * 64:p * 64 + 64, 2, sl],
                                             a[0:D, sl], recs[p][:, sl])

            # pipeline: QKV(jj) | V(jj) | mean(jj-1) | ssq(jj-2); from jj=4
            # pair-0 QK^T slots in (qn(0)/qn(3) are ready) so the ACT exp
            # stream starts while the remaining chunks are still normalizing.
            # QKV/V are emitted BEFORE mean/ssq: the latter wait on the
            # DVE/Pool norm chains, and the in-order PE queue head-of-line
            # blocks the ready QKV matmuls behind them otherwise (~830ns
            # stall per chunk measured).
            # (measured: hoisting the older chunks' lnv/rstd ahead of this
            # chunk's qkr copy in the ACT queue regresses ~1.4us — the
            # delayed copy stalls the next mean matmul more than the
            # earlier rstd helps)
            # (measured: hoisting mean(jj-1) ahead of V(jj) sampled 1.4us
            # worse — V as PE filler between QKV and mean stays)
            for jj in range(6):
                emit_qkv_mm(order[jj])
                emit_qkv_copy(order[jj])
                emit_v(jj)
                if jj >= 1:
                    emit_mean(order[jj - 1])
                if jj >= 2:
                    emit_ssq(order[jj - 2])
                if jj >= 4:
                    emit_qk_t(0, jj - 4)

            # ---- pairs: QK^T -> exp -> AV, cross-pair interleaved ----
            # phase B tail: all remaining norm work BEFORE the first exp
            # (ln/exp table set loaded once), then pair-0 QK^T fills the PE
            # while the last norm chains drain.
            emit_v(6)
            emit_mean(order[5])
            emit_qk_t(0, 2)
            emit_v(7)
            emit_ssq(order[4])
            emit_qk_t(0, 3)
            emit_ssq(order[5])

            # global QK/AV software pipeline across all pairs, AV lagging 4
            # token-chunks behind QK. AV additionally pauses 2 iterations at
            # each pair boundary so the PE has QK work queued while the
            # previous pair's denominator chain (ACT lnd/rec + DVE mul)
            # frees the "av" psum bufs — otherwise the av-tag WAR stalls
            # the in-order PE queue ~3.5us per boundary.
            steps = [(j, t) for j in range(3) for t in range(TC)]
            LAG = 4
            av_at = {}
            for (j, t) in steps:
                av_at[j * TC + t + LAG] = (j, t)
            proj_ps = {}
            # AV emitted before QK within each iteration: when the pair-end
            # denominator ln/exp ops spike the ACT backlog, the next QK
            # stalls on its s-psum WAR; AV-first keeps ready matmuls ahead
            # of that stall in the in-order PE queue. (Measured dead ends:
            # QK-first on the norm-emitting iterations, +2 deferral of the
            # next pair's first AV steps, global AV pause — each looked
            # good in the tile-sim but regressed 1.5-3us on HW.)
            for it in range(4, max(av_at) + 1):
                if it in av_at:
                    emit_av_t(*av_at[it])
                if it < len(steps):
                    emit_qk_t(*steps[it])
            # pre-accumulate proj contractions k=0,1 for the first two
            # output chunks AFTER the last AVs (emitting it earlier blocks
            # the final AV matmuls behind the s-tag WAR on the tail exps);
            # co=2 stays in the tail so the 2-slot "s" rotation can't
            # WAR-deadlock the in-order PE queue.
            for co in range(2):
                p_ps = ps.tile([128, N], F32, tag="s", name=f"p_ps{co}")
                proj_ps[co] = p_ps
                for h5 in range(2):
                    for k in range(2):
                        nc.tensor.matmul(
                            p_ps[:, h5 * 512:(h5 + 1) * 512],
                            wp[:, k, co * 128:(co + 1) * 128],
                            aoT[:, k, h5 * 512:(h5 + 1) * 512],
                            start=(k == 0), stop=False)
            # ---- output projection tail ----
            # h5-major: each 512-col half's k=2 contraction, bias and
            # writeback DMA chains off that half's denominator chain (the
            # tail-norm emits half-major too), so the first half streams
            # out while ACT/DVE still process the second. lo-split inside:
            # rows 0-63 (even head) land one ACT chain earlier than 64-127.
            po = {co: pop.tile([128, N], BF16, tag="po", name=f"po{co}")
                  for co in range(2)}
            for h5 in range(2):
                sl = slice(h5 * 512, (h5 + 1) * 512)
                for lo in (0, 64):
                    for co in range(2):
                        nc.tensor.matmul(
                            proj_ps[co][:, sl],
                            wp[lo:lo + 64, 2, co * 128:(co + 1) * 128],
                            aoT[lo:lo + 64, 2, sl],
                            start=False, stop=(lo == 64))
                for co in range(2):
                    nc.vector.tensor_scalar_add(po[co][:, sl],
                                                proj_ps[co][:, sl],
                                                pbt[:, co:co + 1])
                    eng = nc.sync if (co + h5) % 2 == 0 else nc.scalar
                    eng.dma_start(out=outT[co * 128:(co + 1) * 128, sl],
                                  in_=po[co][:, sl])
            # "av" tag: free once pair-2 attn-out is scaled, so these
            # matmuls need not wait for the first output chunk's bias (the
            # "s"-tag WAR) — only the k=2 contraction truly trails.
            p_ps2 = ps.tile([128, N], F32, tag="av", name="p_ps2")
            po2 = pop.tile([128, N], BF16, tag="po", name="po2")
            for h5 in range(2):
                sl = slice(h5 * 512, (h5 + 1) * 512)
                for k in range(KC):
                    nc.tensor.matmul(
                        p_ps2[:, sl],
                        wp[:, k, 2 * 128:3 * 128],
                        aoT[:, k, sl],
                        start=(k == 0), stop=(k == KC - 1))
                nc.vector.tensor_scalar_add(po2[:, sl], p_ps2[:, sl],
                                            pbt[:, 2:3])
                eng = nc.sync if h5 == 0 else nc.scalar
                eng.dma_start(out=outT[2 * 128:3 * 128, sl], in_=po2[:, sl])

    return nc


def _get_prog():
    global _prog
    if _prog is None:
        _prog = _build()
    return _prog


def _make_in_maps(x, qkv_w, proj_w, proj_b):
    from ml_dtypes import bfloat16

    qkv_wT = np.ascontiguousarray(np.asarray(qkv_w, np.float32).T)
    # pack qkv weights into consumption-ordered contiguous groups:
    # [c0|c3|V|c1|c4|c2|c5]; wr[k, p, m] = qkv_wT[k*128+p, m]
    wr = qkv_wT.reshape(3, 128, 3 * C)
    blocks = []
    for j in (0, 3):
        blocks.append(wr[:, :, j * 128:(j + 1) * 128])
    blocks.append(wr[:, :, 768:1152])           # V
    for j in (1, 4, 2, 5):
        blocks.append(wr[:, :, j * 128:(j + 1) * 128])
    wqp = np.concatenate(
        [b.transpose(1, 0, 2).reshape(128, -1) for b in blocks], axis=1)
    wqp = np.ascontiguousarray(wqp).astype(bfloat16)
    proj_wT = np.ascontiguousarray(np.asarray(proj_w, np.float32).T).astype(bfloat16)
    pb = np.ascontiguousarray(
        np.asarray(proj_b, np.float32).reshape(3, 128).T)
    bd_in = np.zeros((128, 128), np.float32)
    for b0 in (0, 64):
        bd_in[b0:b0 + 64, b0:b0 + 64] = 1.0 / D   # ddof fix in rstd exp bias
    bd_in = bd_in.astype(bfloat16)

    shared = {
        "wqp": wqp, "proj_wT": proj_wT, "pb": pb, "bd_in": bd_in,
    }
    x = np.asarray(x, np.float32)
    return [
        {"xT": np.ascontiguousarray(x[b].T).astype(bfloat16), **shared}
        for b in range(B)
    ]


def run(x, qkv_w, proj_w, proj_b, trace=False):
    from concourse.bass_utils import run_bass_kernel_spmd

    nc = _get_prog()
    in_maps = _make_in_maps(x, qkv_w, proj_w, proj_b)
    res = run_bass_kernel_spmd(
        nc, in_maps, core_ids=list(range(NCORES)), trace=trace)
    out = np.stack(
        [res.results[b]["outT"].astype(np.float32).T for b in range(B)])
    return np.ascontiguousarray(out.astype(np.float32)), res


def kernel(x, qkv_w, proj_w, proj_b):
    out, _ = run(x, qkv_w, proj_w, proj_b)
    return out



# revision 106
# speedup vs baseline: 1.0087x; 1.0087x over previous
"""Trainium2 Bass kernel for nn_Attention_41085657153633.

B=8, N=1024, C=384, H=6, D=64 attention with per-head q/k normalization
(mean/std over head_dim, ddof=1), softmax(QK^T/sqrt(D)) @ V, output proj.

Sharding: data-parallel over B — each of the 8 NeuronCores handles one
batch element end-to-end (no collectives).

Per-core dataflow (all matmul operands bf16; fp32 PSUM accumulation):
  - host supplies x[b]^T, qkv_w^T, proj_w^T pre-cast to bf16 (halves HBM
    traffic, enables fast weight load on the PE).
  - QKV^T computed head-major for Q,K ([d on partitions, tokens on free]
    — the layout QK^T wants) and token-major for V (the AV stationary).
    V is stored as [v_h | 64 ones cols] per head so AV rows 64-127
    accumulate the softmax denominator broadcast across 64 partitions
    for free in the matmul M dimension.
  - q/k normalization over the partition (d) axis via block-diagonal
    ones matmuls: mean_bcast = blockdiag(1/64) @ q and ssq_bcast =
    blockdiag(1/64) @ (q-mean)^2 come out broadcast across each head's
    64 partitions by construction; rstd = exp(-0.5*ln(ssq) + b) with the
    ddof=1 correction folded into the ACT bias (ln/exp share one ACT
    table set with the softmax exp — zero table switches after load).
  - scores S^T[s,t] per head via K^T x Q^T (contraction over d=64);
    head pairs occupy PE row groups T0/T8, alternating so each weight
    load overlaps the other group's matmul.
  - softmax denominators inverted as exp(-ln d) on ACT (ln/exp/softmax
    exp share one table set — no table switches in the whole kernel).
  - scheduling: the norm pipeline is software-pipelined into the QKV/V
    matmul stream (pair-0 QK^T already slots into the last chunk
    iterations), and one global QK/AV pipeline runs across all pairs
    with AV lagging 4 token-chunks behind QK and emitted BEFORE QK per
    iteration (keeps ready matmuls ahead of the s-psum WAR stall when
    the pair-end denominator ln/exp ops spike the ACT backlog). ACT is
    the phase-C metronome (~80us of exp/ln); per-head denominator
    chains are emitted right after that head's last AV matmul so the
    "av" psum bufs free one at a time. proj contractions k=0,1
    pre-accumulate after the AV stream, overlapping the tail exps +
    pair-2 denominators. PE idle gaps reset the p-state ramp (0.65/1.2
    GHz until 3us continuous busy, 2.4GHz after), so the stream order
    is tuned to keep the PE queue head unblocked: QKV/V before the
    chained mean/ssq per phase-B iteration, warmup matmuls covering
    the input-DMA window.
  - input DMA: qkv weights host-packed into consumption-ordered
    contiguous groups (one contiguous run per partition per DMA; the
    naive strided column slices produced 256B packets), x^T split
    per contraction chunk, outputs written back per 512-col half on
    alternating queues.
  - qk_ps -> SBUF copies run on ACT (Copy lives in every table set);
    ACT otherwise idles until the first norm ln at ~21us.
  - softmax max-subtraction skipped: |S/8| <~ 7, exp stays in range.
  - output proj from attn_out^T; bf16 result DMA'd out, host casts.

Measured on trn2 (ntff profile): 121.0us vs the 148.0/130.4us session
baseline. Engine budget: PE matmul ~87us union-busy (p-state sensitive),
ACT ~81us (48 softmax-exp tiles at 1.07ns/col + norm/denominator
ln/exp), DVE ~50us, Pool idle. Paths measured and rejected: DVE
InstReciprocal for denominators (6.4ns/col on HW), fp8e4 DoubleRow for
QK^T/AV (2x PE rate but 2.2-3.9e-2 rel err, over the 2e-2 gate),
token-major K + folded per-key exp scale (ACT -6.4us but +8us PE/DVE at
phase-B clocks), custom-DVE ops and gpsimd divide/PSUM access (rejected
by this walrus build).
"""

import sys

sys.path.insert(0, "/opt/trn_rl_repo")

import json

import numpy as np

B, N, C = 8, 1024, 384
H, D = 6, 64
NCORES = 8

_prog = None


def _install_multiwait_fixup():
    """This container's walrus build rejects >1 sync wait per instruction
    ("Too many sync wait commands"). Rewrite the BIR JSON before compile:
    hoist extra waits onto single-wait EventSemaphore instructions
    inserted just before the owner on the same engine (engines dispatch
    in program order, so the gating is preserved)."""
    from concourse import bass2jax, bass_utils

    if getattr(bass_utils, "_multiwait_fixup", False):
        return
    bass_utils._multiwait_fixup = True

    orig = bass_utils.compile_bir_kernel

    def _split(bir_json: bytes) -> bytes:
        j = json.loads(bir_json)
        for fn in j.get("functions", []):
            for bb in fn.get("blocks", []):
                out = []
                for inst in bb.get("instructions", []):
                    si = inst.get("sync_info")
                    waits = si.get("on_wait", []) if si else []
                    if len(waits) > 1:
                        for k, w in enumerate(waits[:-1]):
                            out.append({
                                "debug": inst.get("debug", 0),
                                "engine": inst["engine"],
                                "ins": [],
                                "outs": [],
                                "name": f"{inst['name']}-sw{k}",
                                "opcode": "EventSemaphore",
                                "sync_info": {"on_update": [], "on_wait": [w]},
                            })
                        si["on_wait"] = [waits[-1]]
                    out.append(inst)
                bb["instructions"] = out
        return json.dumps(j).encode()

    def patched(bir_json, tmpdir, neff_name="file.neff"):
        return orig(_split(bir_json), tmpdir, neff_name)

    bass_utils.compile_bir_kernel = patched
    bass2jax.compile_bir_kernel = patched
    # (note: --enable-ldw-opt=true was tried and is rejected by this
    # walrus build on our BIR — keep the default false)


def _build():
    import concourse.bass as bass
    import concourse.tile as tile
    from concourse import mybir

    _install_multiwait_fixup()

    F32 = mybir.dt.float32
    BF16 = mybir.dt.bfloat16
    EXP = mybir.ActivationFunctionType.Exp
    LN = mybir.ActivationFunctionType.Ln
    COPY = mybir.ActivationFunctionType.Copy
    SQUARE = mybir.ActivationFunctionType.Square

    nc = bass.Bass("TRN2")
    xT = nc.dram_tensor("xT", [C, N], BF16, kind="ExternalInput")
    # qkv weights host-packed into consumption-ordered contiguous groups
    # [c0|c3|V|c1|c4|c2|c5] so each input DMA moves one contiguous run per
    # partition (the old strided column slices produced 256B DMA packets)
    wqp = nc.dram_tensor("wqp", [128, 3456], BF16, kind="ExternalInput")
    proj_wT = nc.dram_tensor("proj_wT", [C, C], BF16, kind="ExternalInput")
    pb = nc.dram_tensor("pb", [128, 3], F32, kind="ExternalInput")
    bd_in = nc.dram_tensor("bd_in", [128, 128], BF16, kind="ExternalInput")
    outT = nc.dram_tensor("outT", [C, N], BF16, kind="ExternalOutput")

    KC = C // 128   # 3 contraction chunks of the model dim
    TC = N // 128   # 8 token chunks
    scale = float(D) ** -0.5
    order = [0, 3, 1, 4, 2, 5]   # q0,k0,q1,k1,q2,k2 chunk emission order

    with tile.TileContext(nc) as tc:
      with nc.allow_low_precision(reason="bf16 matmul intermediates"):
        with tc.tile_pool(name="consts", bufs=1) as consts, \
             tc.tile_pool(name="ins", bufs=1) as ins, \
             tc.tile_pool(name="persist", bufs=1) as persist, \
             tc.tile_pool(name="work", bufs=2) as work, \
             tc.tile_pool(name="es", bufs=16) as esp, \
             tc.tile_pool(name="avn", bufs=2) as avn, \
             tc.tile_pool(name="po", bufs=3) as pop, \
             tc.tile_pool(name="ps", bufs=2, space="PSUM") as ps:

            # ---- constants + input DMA ----
            bd = consts.tile([128, 128], BF16)
            pbt = consts.tile([128, 3], F32)
            warm_w = consts.tile([128, 64], BF16)
            ddof_b = consts.tile([128, 1], F32)
            warm_act = consts.tile([128, 1], F32)
            nc.vector.memset(warm_w[:], 0.5)
            nc.vector.memset(ddof_b[:], -0.5 * float(np.log(64.0 / 63.0)))

            xt = ins.tile([128, KC, N], BF16)
            wq = ins.tile([128, 3456], BF16)
            wp = ins.tile([128, KC, C], BF16)
            # group offsets in the packed wq layout
            OFFC = {0: 0, 3: 384, 1: 1920, 4: 2304, 2: 2688, 5: 3072}
            OFFV = 768

            # HAM warmup gated only on the tiny ddof memset — no DMA
            # dependency, so the PE clock gate starts opening immediately.
            warm_ps = ps.tile([64, 512], F32, tag="s")
            warm_rhs = bass.AP(
                tensor=warm_w.tensor, offset=warm_w.offset,
                ap=[list(warm_w.ap[0]), [0, 8], [1, 64]])  # [128,8,64] step-0
            for _ in range(12):
                nc.tensor.matmul(warm_ps[:], warm_w[:], warm_rhs,
                                 start=True, stop=True)

            xr = xT.rearrange("(k p) n -> p k n", p=128)
            # the first QKV matmul needs xt k0 AND the c0 weight group:
            # put each at the head of a different DMA queue so they land
            # in parallel as early as possible. Remaining chunks follow in
            # consumption order, all host-packed contiguous.
            nc.sync.dma_start(out=xt[:, 0, :], in_=xr[:, 0, :])
            nc.scalar.dma_start(out=wq[:, 0:384], in_=wqp[:, 0:384])    # c0
            nc.scalar.dma_start(out=xt[:, 1, :], in_=xr[:, 1, :])
            nc.scalar.dma_start(out=xt[:, 2, :], in_=xr[:, 2, :])
            # dummy exp AFTER the DMA triggers (same Scalar queue): pulls
            # the single ln/exp ACT table load into the input-DMA wait
            # without delaying the DMA kickoff
            nc.scalar.activation(warm_act[:], ddof_b[:], EXP)
            nc.sync.dma_start(out=wq[:, 384:768], in_=wqp[:, 384:768])  # c3
            nc.sync.dma_start(out=bd[:], in_=bd_in[:, :])
            nc.sync.dma_start(out=wq[:, 768:1920], in_=wqp[:, 768:1920])  # V
            nc.sync.dma_start(out=pbt[:], in_=pb[:, :])
            nc.sync.dma_start(out=wq[:, 1920:3456], in_=wqp[:, 1920:3456])
            nc.sync.dma_start(
                out=wp[:], in_=proj_wT.rearrange("(k p) m -> p k m", p=128))

            vo = persist.tile([128, TC, H, 128], BF16)
            nc.vector.memset(vo[:, :, :, D:128], 1.0)

            qn = persist.tile([128, 2 * H, N], BF16)   # normalized q | raw k
            aoT = persist.tile([128, KC, N], BF16)     # attn out (proj moving)

            # ---- phase B emitters: QKV chunks + pipelined normalization ----
            qk_ps = {}      # chunk j -> psum tile with raw q|k rows
            qkr = {}        # chunk j -> bf16 copy in SBUF
            mean_ps = {}
            qc = {}
            qc2 = {}
            ssq_ps = {}

            def emit_qkv_mm(j):
                p = ps.tile([128, N], F32, tag="s", name=f"qk_ps{j}")
                qk_ps[j] = p
                for k in range(KC):
                    for h5 in range(2):
                        nc.tensor.matmul(
                            p[:, h5 * 512:(h5 + 1) * 512],
                            wq[:, OFFC[j] + k * 128:OFFC[j] + (k + 1) * 128],
                            xt[:, k, h5 * 512:(h5 + 1) * 512],
                            start=(k == 0), stop=(k == KC - 1))

            def emit_qkv_copy(j):
                # PSUM->SBUF bf16 copy on ACT: ACT idles ~10us at the start
                # of phase B while DVE is the loaded engine there; Copy is
                # in every ACT table set. (Measured: moving the last three
                # chunks' copies to DVE to unblock the pair-0 lnv/rstd
                # sampled 2us worse — keep all six on ACT.)
                q = work.tile([128, N], BF16, tag="qkr", name=f"qkr{j}")
                nc.scalar.activation(q[:], qk_ps[j][:], COPY)
                qkr[j] = q

            def emit_v(t):
                v_ps = ps.tile([128, C], F32, tag="s", name=f"v_ps{t}")
                for k in range(KC):
                    nc.tensor.matmul(
                        v_ps[:],
                        xt[:, k, t * 128:(t + 1) * 128],
                        wq[:, OFFV + k * 384:OFFV + (k + 1) * 384],
                        start=(k == 0), stop=(k == KC - 1))
                nc.vector.tensor_copy(
                    vo[:, t, :, 0:D],
                    v_ps[:].rearrange("p (h d) -> p h d", h=H))

            def emit_mean(j):
                """mean broadcast + centered q, squared (DVE chain)."""
                m = ps.tile([128, N], F32, tag="av", name=f"mean_ps{j}")
                mean_ps[j] = m
                for h5 in range(2):
                    nc.tensor.matmul(m[:, h5 * 512:(h5 + 1) * 512],
                                     bd[:], qkr[j][:, h5 * 512:(h5 + 1) * 512],
                                     start=True, stop=True)
                c = work.tile([128, N], BF16, tag="qc", name=f"qc{j}")
                nc.vector.tensor_sub(c[:], qkr[j][:], m[:])
                qc[j] = c
                c2 = work.tile([128, N], BF16, tag="qc2", name=f"qc2{j}")
                nc.vector.tensor_mul(c2[:], c[:], c[:])
                qc2[j] = c2

            def emit_ssq(j):
                """ssq broadcast -> rstd (ACT ln/exp; Rsqrt is blocked in
                this bass build) -> qn (DVE mul)."""
                s = ps.tile([128, N], F32, tag="av", name=f"ssq_ps{j}")
                ssq_ps[j] = s
                for h5 in range(2):
                    nc.tensor.matmul(s[:, h5 * 512:(h5 + 1) * 512],
                                     bd[:], qc2[j][:, h5 * 512:(h5 + 1) * 512],
                                     start=True, stop=True)
                lnv = work.tile([128, N], F32, tag="lnv", name=f"lnv{j}")
                nc.scalar.activation(lnv[:], s[:], LN)
                rstd = work.tile([128, N], BF16, tag="rstd", name=f"rstd{j}")
                nc.scalar.activation(rstd[:], lnv[:], EXP, scale=-0.5,
                                     bias=ddof_b[:])
                nc.vector.tensor_mul(qn[:, j, :], qc[j][:], rstd[:])

            es_tiles = {}   # (pair, t, p) -> bf16 exp(scores) tile

            def emit_qk_t(j, t):
                """Scores S^T for heads 2j,2j+1, token chunk t (64-row PE
                mode, groups T0/T8 alternating) + the exp on ACT."""
                s_tiles = {}
                for p in range(2):
                    s_tiles[p] = ps.tile([128, N], F32, tag="s",
                                         name=f"s{j}_{t}_{p}")
                for h5 in range(2):
                    for p in range(2):
                        lo = p * 64
                        nc.tensor.matmul(
                            s_tiles[p][:, h5 * 512:(h5 + 1) * 512],
                            qn[lo:lo + 64, 3 + j, t * 128:(t + 1) * 128],
                            qn[lo:lo + 64, j, h5 * 512:(h5 + 1) * 512],
                            start=True, stop=True)
                for p in range(2):
                    es = esp.tile([128, N], BF16, tag="es",
                                  name=f"es{j}_{t}_{p}")
                    nc.scalar.activation(es[:], s_tiles[p][:], EXP, scale=scale)
                    es_tiles[(j, t, p)] = es

            av_ps = {}
            av_src = {}

            def emit_av_t(j, t):
                """AV accumulation step t for pair j (128-row mode). On the
                final step, each head's denominator/scale chain is emitted
                right after that head's last matmul so av_ps bufs free one
                at a time (halves the pair-boundary PE stall on the "av"
                tag WAR)."""
                for p in range(2):
                    if t == 0:
                        av_ps[(j, p)] = ps.tile([128, N], F32, tag="av",
                                                name=f"av{j}_{p}")
                    for h5 in range(2):
                        nc.tensor.matmul(
                            av_ps[(j, p)][:, h5 * 512:(h5 + 1) * 512],
                            vo[:, t, 2 * j + p, :],
                            es_tiles[(j, t, p)][:, h5 * 512:(h5 + 1) * 512],
                            start=(t == 0), stop=(t == TC - 1))
                    if t == TC - 1 and j < 2:
                        emit_norm_out_p(j, p)
                if t == TC - 1 and j == 2:
                    emit_norm_out_tail()

            def emit_norm_out_p(j, p):
                """Softmax denominator reciprocal. Rows 64-127 hold the
                denominator already broadcast across 64 partitions (ones
                columns of vo).

                Pairs 0,1: DVE InstReciprocal (6.4ns/col — slow, but fully
                off the critical path: the avc copy already freed the psum,
                and aoT is only needed by the proj tail). Putting these on
                ACT instead injects 4.2us into the ACT exp backlog, which
                starves the PE's QK stream mid-flight (measured in sim).
                Pair 2: ACT ln/exp — at the kernel tail ACT is idle and its
                2.2us latency beats DVE's 6.5us."""
                a = av_src.get((j, p), av_ps[(j, p)])
                rec = avn.tile([64, N], F32, tag="rec", name=f"rec{j}{p}")
                lnd = avn.tile([64, N], F32, tag="lnd", name=f"lnd{j}{p}")
                lo = p * 64
                nc.scalar.activation(lnd[:], a[D:128, :], LN)
                nc.scalar.activation(rec[:], lnd[:], EXP, scale=-1.0)
                nc.vector.tensor_mul(aoT[lo:lo + 64, j, :],
                                     a[0:D, :], rec[:])

            def emit_norm_out_tail():
                """Pair 2 closes the kernel: both heads' denominator chains
                per 512-col half, half-major order, so the k=2 proj matmuls
                and po2 bias (already h5/lo-split) start on the first half
                while ACT still processes the second."""
                recs = {}
                lnds = {}
                for p in range(2):
                    recs[p] = avn.tile([64, N], F32, tag="rec",
                                       name=f"rec2{p}")
                    lnds[p] = avn.tile([64, N], F32, tag="lnd",
                                       name=f"lnd2{p}")
                for h5 in range(2):
                    sl = slice(h5 * 512, (h5 + 1) * 512)
                    for p in range(2):
                        a = av_ps[(2, p)]
                        nc.scalar.activation(lnds[p][:, sl], a[D:128, sl], LN)
                        nc.scalar.activation(recs[p][:, sl], lnds[p][:, sl],
                                             EXP, scale=-1.0)
                        nc.vector.tensor_mul(aoT[p * 64:p * 64 + 64, 2, sl],
                                             a[0:D, sl], recs[p][:, sl])

            # pipeline: QKV(jj) | V(jj) | mean(jj-1) | ssq(jj-2); from jj=4
            # pair-0 QK^T slots in (qn(0)/qn(3) are ready) so the ACT exp
            # stream starts while the remaining chunks are still normalizing.
            # QKV/V are emitted BEFORE mean/ssq: the latter wait on the
            # DVE/Pool norm chains, and the in-order PE queue head-of-line
            # blocks the ready QKV matmuls behind them otherwise (~830ns
            # stall per chunk measured).
            # (measured: hoisting the older chunks' lnv/rstd ahead of this
            # chunk's qkr copy in the ACT queue regresses ~1.4us — the
            # delayed copy stalls the next mean matmul more than the
            # earlier rstd helps)
            # (measured: hoisting mean(jj-1) ahead of V(jj) sampled 1.4us
            # worse — V as PE filler between QKV and mean stays)
            for jj in range(6):
                emit_qkv_mm(order[jj])
                emit_qkv_copy(order[jj])
                emit_v(jj)
                if jj >= 1:
                    emit_mean(order[jj - 1])
                if jj >= 2:
                    emit_ssq(order[jj - 2])
                if jj >= 4:
                    emit_qk_t(0, jj - 4)

            # ---- pairs: QK^T -> exp -> AV, cross-pair interleaved ----
            # phase B tail: all remaining norm work BEFORE the first exp
            # (ln/exp table set loaded once), then pair-0 QK^T fills the PE
            # while the last norm chains drain.
            emit_v(6)
            emit_mean(order[5])
            emit_qk_t(0, 2)
            emit_v(7)
            emit_ssq(order[4])
            emit_qk_t(0, 3)
            emit_ssq(order[5])

            # global QK/AV software pipeline across all pairs, AV lagging 4
            # token-chunks behind QK. AV additionally pauses 2 iterations at
            # each pair boundary so the PE has QK work queued while the
            # previous pair's denominator chain (ACT lnd/rec + DVE mul)
            # frees the "av" psum bufs — otherwise the av-tag WAR stalls
            # the in-order PE queue ~3.5us per boundary.
            steps = [(j, t) for j in range(3) for t in range(TC)]
            LAG = 4
            av_at = {}
            for (j, t) in steps:
                av_at[j * TC + t + LAG] = (j, t)
            proj_ps = {}
            # AV emitted before QK within each iteration: when the pair-end
            # denominator ln/exp ops spike the ACT backlog, the next QK
            # stalls on its s-psum WAR; AV-first keeps ready matmuls ahead
            # of that stall in the in-order PE queue. (Measured dead ends:
            # QK-first on the norm-emitting iterations, +2 deferral of the
            # next pair's first AV steps, global AV pause — each looked
            # good in the tile-sim but regressed 1.5-3us on HW.)
            for it in range(4, max(av_at) + 1):
                if it in av_at:
                    emit_av_t(*av_at[it])
                if it < len(steps):
                    emit_qk_t(*steps[it])
            # pre-accumulate proj contractions k=0,1 for the first two
            # output chunks AFTER the last AVs (emitting it earlier blocks
            # the final AV matmuls behind the s-tag WAR on the tail exps);
            # co=2 stays in the tail so the 2-slot "s" rotation can't
            # WAR-deadlock the in-order PE queue.
            for co in range(2):
                p_ps = ps.tile([128, N], F32, tag="s", name=f"p_ps{co}")
                proj_ps[co] = p_ps
                for h5 in range(2):
                    for k in range(2):
                        nc.tensor.matmul(
                            p_ps[:, h5 * 512:(h5 + 1) * 512],
                            wp[:, k, co * 128:(co + 1) * 128],
                            aoT[:, k, h5 * 512:(h5 + 1) * 512],
                            start=(k == 0), stop=False)
            # ---- output projection tail ----
            # h5-major: each 512-col half's k=2 contraction, bias and
            # writeback DMA chains off that half's denominator chain (the
            # tail-norm emits half-major too), so the first half streams
            # out while ACT/DVE still process the second. lo-split inside:
            # rows 0-63 (even head) land one ACT chain earlier than 64-127.
            po = {co: pop.tile([128, N], BF16, tag="po", name=f"po{co}")
                  for co in range(2)}
            for h5 in range(2):
                sl = slice(h5 * 512, (h5 + 1) * 512)
                for lo in (0, 64):
                    for co in range(2):
                        nc.tensor.matmul(
                            proj_ps[co][:, sl],
                            wp[lo:lo + 64, 2, co * 128:(co + 1) * 128],
                            aoT[lo:lo + 64, 2, sl],
                            start=False, stop=(lo == 64))
                for co in range(2):
                    nc.vector.tensor_scalar_add(po[co][:, sl],
                                                proj_ps[co][:, sl],
                                                pbt[:, co:co + 1])
                    eng = nc.sync if (co + h5) % 2 == 0 else nc.scalar
                    eng.dma_start(out=outT[co * 128:(co + 1) * 128, sl],
                                  in_=po[co][:, sl])
            # "av" tag: free once pair-2 attn-out is scaled, so these
            # matmuls need not wait for the first output chunk's bias (the
            # "s"-tag WAR) — only the k=2 contraction truly trails.
            p_ps2 = ps.tile([128, N], F32, tag="av", name="p_ps2")
            po2 = pop.tile([128, N], BF16, tag="po", name="po2")
            for h5 in range(2):
                sl = slice(h5 * 512, (h5 + 1) * 512)
                for k in range(KC):
                    nc.tensor.matmul(
                        p_ps2[:, sl],
                        wp[:, k, 2 * 128:3 * 128],
                        aoT[:, k, sl],
                        start=(k == 0), stop=(k == KC - 1))
                nc.vector.tensor_scalar_add(po2[:, sl], p_ps2[:, sl],
                                            pbt[:, 2:3])
                eng = nc.sync if h5 == 0 else nc.scalar
                eng.dma_start(out=outT[2 * 128:3 * 128, sl], in_=po2[:, sl])

    return nc


def _get_prog():
    global _prog
    if _prog is None:
        _prog = _build()
    return _prog


def _make_in_maps(x, qkv_w, proj_w, proj_b):
    from ml_dtypes import bfloat16

    qkv_wT = np.ascontiguousarray(np.asarray(qkv_w, np.float32).T)
    # pack qkv weights into consumption-ordered contiguous groups:
    # [c0|c3|V|c1|c4|c2|c5]; wr[k, p, m] = qkv_wT[k*128+p, m]
    wr = qkv_wT.reshape(3, 128, 3 * C)
    blocks = []
    for j in (0, 3):
        blocks.append(wr[:, :, j * 128:(j + 1) * 128])
    blocks.append(wr[:, :, 768:1152])           # V
    for j in (1, 4, 2, 5):
        blocks.append(wr[:, :, j * 128:(j + 1) * 128])
    wqp = np.concatenate(
        [b.transpose(1, 0, 2).reshape(128, -1) for b in blocks], axis=1)
    wqp = np.ascontiguousarray(wqp).astype(bfloat16)
    proj_wT = np.ascontiguousarray(np.asarray(proj_w, np.float32).T).astype(bfloat16)
    pb = np.ascontiguousarray(
        np.asarray(proj_b, np.float32).reshape(3, 128).T)
    bd_in = np.zeros((128, 128), np.float32)
    for b0 in (0, 64):
        bd_in[b0:b0 + 64, b0:b0 + 64] = 1.0 / D   # ddof fix in rstd exp bias
    bd_in = bd_in.astype(bfloat16)

    shared = {
        "wqp": wqp, "proj_wT": proj_wT, "pb": pb, "bd_in": bd_in,
    }
    x = np.asarray(x, np.float32)
    return [
        {"xT": np.ascontiguousarray(x[b].T).astype(bfloat16), **shared}
        for b in range(B)
    ]


def run(x, qkv_w, proj_w, proj_b, trace=False):
    from concourse.bass_utils import run_bass_kernel_spmd

    nc = _get_prog()
    in_maps = _make_in_maps(x, qkv_w, proj_w, proj_b)
    res = run_bass_kernel_spmd(
        nc, in_maps, core_ids=list(range(NCORES)), trace=trace)
    out = np.stack(
        [res.results[b]["outT"].astype(np.float32).T for b in range(B)])
    return np.ascontiguousarray(out.astype(np.float32)), res


def kernel(x, qkv_w, proj_w, proj_b):
    out, _ = run(x, qkv_w, proj_w, proj_b)
    return out

